# revision 4
# baseline (speedup 1.0000x reference)
"""MultiHeadAttention Trainium2 Bass kernel (8-core SPMD).

Reference computes (out, scores) where
  q = x @ WQ.T + bQ ; k = x @ WK.T + bK ; v = k   (source quirk: V == K)
  scores = softmax(mask + q k^T / sqrt(dh))       # [B, nh, S, S]  ~1 GiB f32
  out = (scores @ v) proj WO + bO                 # [B, S, H]

Sharding: batch x query-block. Core c handles batch b=c//4 and query rows
r=(c%4)*1024 .. +1024, all 8 heads. No cross-core communication.

Per-core device pipeline (all matmuls float32r, full PE rate at N=512):
  xT (hidden-major x) -> KT [d, keys] and K-aug [keys, d-grouped+ones-col]
  and QT [d, q] (pre-scaled by 1/sqrt(dh)).
  Natural side:    scores[q,k] -> exp (fused accum denominator) -> normalize
                   -> DMA out (the 1 GiB scores output, q-major).
  Transposed side: scores^T[k,q] -> exp -> AV matmul with K-aug as the
                   stationary operand; the appended ones-column yields the
                   transposed-side softmax denominators in the same psum.
  avT normalized via a DRAM-bounced partition-broadcast of 1/denom, then
  out^T = WOT.T @ avT + bO, DMAed back; host transposes.
"""

import sys

sys.path.insert(0, "/opt/trn_rl_repo")

from contextlib import ExitStack

import numpy as np

import concourse.bass as bass
import concourse.mybir as mybir
import concourse.tile as tile

B, S, H, NH, DH = 2, 4096, 512, 8, 64
NCORES = 8
QL = S // 4  # local query rows per core (1024)
SCALE = 1.0 / np.sqrt(DH)

f32 = mybir.dt.float32
f32r = mybir.dt.float32r
AF = mybir.ActivationFunctionType
AX = mybir.AxisListType

LAST_EXEC_NS = None


# --- walrus in this toolchain encodes at most ONE sem-wait per instruction;
# move extra waits onto single-wait NoOps inserted before, same engine. ---
def _legalize_single_wait(nc):
    ctr = 0
    for f in nc.m.functions:
        for bb in f.blocks:
            out = []
            changed = False
            for inst in bb.instructions:
                si = inst.sync_info
                waits = list(si.on_wait) if (si and si.on_wait) else []
                ups = list(si.on_update) if (si and si.on_update) else []
                if len(ups) > 1:
                    raise RuntimeError(f"{inst.name}: {len(ups)} sem updates")
                if len(waits) > 1:
                    changed = True
                    for w in waits[:-1]:
                        ctr += 1
                        out.append(
                            mybir.InstNoOp(
                                name=f"__legalize_wait_nop_{ctr}",
                                engine=inst.engine,
                                sync_info=mybir.SyncInfo(on_wait=[w], on_update=[]),
                            )
                        )
                    inst.sync_info = mybir.SyncInfo(on_wait=waits[-1:], on_update=ups)
                out.append(inst)
            if changed:
                try:
                    bb.instructions = out
                except Exception:
                    bb.instructions.clear()
                    for i in out:
                        bb.instructions.append(i)


def _bcast_rows(dram_ap, nrows):
    """Partition-broadcast AP: replicate a 1D DRAM row across nrows partitions."""
    return bass.AP(
        tensor=dram_ap.tensor, offset=dram_ap.offset, ap=[[0, nrows]] + dram_ap.ap
    )


def build_kernel(has_mask: bool):
    nc = bass.Bass()

    xt_e = nc.declare_dram_parameter("xt", [H, S], f32, isOutput=False)
    xq_e = nc.declare_dram_parameter("xq", [H, QL], f32, isOutput=False)
    wqt_e = nc.declare_dram_parameter("wqt", [H, H], f32, isOutput=False)
    wkt_e = nc.declare_dram_parameter("wkt", [H, H], f32, isOutput=False)
    wot_e = nc.declare_dram_parameter("wot", [H, H], f32, isOutput=False)
    bqs_e = nc.declare_dram_parameter("bqs", [H, 1], f32, isOutput=False)
    bk_e = nc.declare_dram_parameter("bk", [H, 1], f32, isOutput=False)
    bo_e = nc.declare_dram_parameter("bo", [H, 1], f32, isOutput=False)
    bkb_e = nc.declare_dram_parameter("bkb", [128, H], f32, isOutput=False)
    if has_mask:
        maskt_e = nc.declare_dram_parameter("maskt", [S, 1], f32, isOutput=False)
        maskb_e = nc.declare_dram_parameter("maskb", [128, S], f32, isOutput=False)

    scores_e = nc.declare_dram_parameter("scores", [NH, QL, S], f32, isOutput=True)
    outt_e = nc.declare_dram_parameter("outt", [H, QL], f32, isOutput=True)
    # spill space (device-internal scratch; declared as outputs because the
    # PJRT path only materializes IO tensors)
    ktsp_e = nc.declare_dram_parameter("ktsp", [2, 128, S], f32, isOutput=True)
    kasp_e = nc.declare_dram_parameter("kasp", [32, 128, 8 * 65], f32, isOutput=True)
    qtsp_e = nc.declare_dram_parameter("qtsp", [2, 128, QL], f32, isOutput=True)
    dsp_e = nc.declare_dram_parameter("dsp", [NH, QL], f32, isOutput=True)

    with tile.TileContext(nc) as tc, ExitStack() as ctx:
        consts = ctx.enter_context(tc.tile_pool(name="consts", bufs=1))
        ktp = ctx.enter_context(tc.tile_pool(name="ktp", bufs=2))
        kap = ctx.enter_context(tc.tile_pool(name="kap", bufs=32))
        qtp = ctx.enter_context(tc.tile_pool(name="qtp", bufs=2))
        prp = ctx.enter_context(tc.tile_pool(name="prp", bufs=2))
        etp = ctx.enter_context(tc.tile_pool(name="etp", bufs=3))
        avp = ctx.enter_context(tc.tile_pool(name="avp", bufs=4))
        bcp = ctx.enter_context(tc.tile_pool(name="bcp", bufs=1))
        dnp = ctx.enter_context(tc.tile_pool(name="dnp", bufs=1))
        otp = ctx.enter_context(tc.tile_pool(name="otp", bufs=2))
        accp = ctx.enter_context(tc.tile_pool(name="accp", bufs=2))
        stp = ctx.enter_context(tc.tile_pool(name="stp", bufs=2))
        xtp = ctx.enter_context(tc.tile_pool(name="xtp", bufs=6))
        psp = ctx.enter_context(tc.tile_pool(name="psp", bufs=4, space="PSUM"))

        # ---- constants ----
        wkt_sb, wqt_sb, wot_sb = [], [], []
        for j in range(4):
            w = consts.tile([128, H], f32r, tag=f"wkt{j}")
            nc.sync.dma_start(out=w[:], in_=wkt_e[j * 128:(j + 1) * 128, :].bitcast(f32r))
            wkt_sb.append(w)
        for j in range(4):
            w = consts.tile([128, H], f32r, tag=f"wqt{j}")
            nc.sync.dma_start(out=w[:], in_=wqt_e[j * 128:(j + 1) * 128, :].bitcast(f32r))
            wqt_sb.append(w)
        for j in range(4):
            w = consts.tile([128, H], f32r, tag=f"wot{j}")
            nc.sync.dma_start(out=w[:], in_=wot_e[j * 128:(j + 1) * 128, :].bitcast(f32r))
            wot_sb.append(w)
        bqs_sb, bk_sb, bo_sb = [], [], []
        for m in range(4):
            t = consts.tile([128, 1], f32, tag=f"bqs{m}")
            nc.gpsimd.dma_start(out=t[:], in_=bqs_e[m * 128:(m + 1) * 128, :])
            bqs_sb.append(t)
            t = consts.tile([128, 1], f32, tag=f"bk{m}")
            nc.gpsimd.dma_start(out=t[:], in_=bk_e[m * 128:(m + 1) * 128, :])
            bk_sb.append(t)
            t = consts.tile([128, 1], f32, tag=f"bo{m}")
            nc.gpsimd.dma_start(out=t[:], in_=bo_e[m * 128:(m + 1) * 128, :])
            bo_sb.append(t)
        bkb_sb = consts.tile([128, H], f32, tag="bkb")
        nc.gpsimd.dma_start(out=bkb_sb[:], in_=bkb_e[:])
        ones_f = consts.tile([128, 8], f32, tag="ones_f")
        nc.vector.memset(ones_f[:], 1.0)
        ones_r = consts.tile([128, 8], f32r, tag="ones_r")
        nc.vector.tensor_copy(ones_r[:], ones_f[:])
        if has_mask:
            maskt_sb = consts.tile([128, 32], f32, tag="maskt")
            nc.gpsimd.dma_start(
                out=maskt_sb[:],
                in_=maskt_e[:, 0].rearrange("(j p) -> p j", p=128),
            )
            maskb_sb = consts.tile([128, S], f32, tag="maskb")
            nc.gpsimd.dma_start(out=maskb_sb[:], in_=maskb_e[:])

        # ---- long-lived tensors ----
        kt_sb = [None] * 4   # KT m-tiles [128 d, S keys] f32r (m>=2 spilled)
        qt_sb = [None] * 4   # QT m-tiles [128 d, QL] f32r
        ka_sb = [None] * 32  # K-aug key-tiles [128 keys, 8*65] (split by head half)
        kt_sb[0] = ktp.tile([128, S], f32r, tag="kt", name="kt0")
        kt_sb[1] = ktp.tile([128, S], f32r, tag="kt", name="kt1")
        qt_sb[0] = qtp.tile([128, QL], f32r, tag="qt", name="qt0")
        qt_sb[1] = qtp.tile([128, QL], f32r, tag="qt", name="qt1")
        avt_sb = [avp.tile([128, QL], f32r, tag="avt", name=f"avt{i}") for i in range(4)]

        def g3(ap, c):
            return ap.rearrange("p (h c) -> p h c", c=c)

        # ---- phase P: projections ----
        for blk in range(8):
            xt_sb = []
            for j in range(4):
                t = xtp.tile([128, 512], f32r, tag="xt")
                nc.sync.dma_start(
                    out=t[:],
                    in_=xt_e[j * 128:(j + 1) * 128,
                             blk * 512:(blk + 1) * 512].bitcast(f32r),
                )
                xt_sb.append(t)
            # KT[d, keys]: lhsT = WKT[:, d-slice], rhs = xT block
            for m in range(4):
                pk = psp.tile([128, 512], f32, tag="ps")
                for j in range(4):
                    nc.tensor.matmul(
                        pk[:], wkt_sb[j][:, m * 128:(m + 1) * 128], xt_sb[j][:],
                        start=(j == 0), stop=(j == 3), skip_group_check=True,
                    )
                if m < 2:
                    nc.vector.tensor_scalar_add(
                        kt_sb[m][:, blk * 512:(blk + 1) * 512], pk[:], bk_sb[m][:]
                    )
                else:
                    st = stp.tile([128, 512], f32r, tag="st")
                    nc.vector.tensor_scalar_add(st[:], pk[:], bk_sb[m][:])
                    nc.sync.dma_start(
                        out=ktsp_e[m - 2, :, blk * 512:(blk + 1) * 512].bitcast(f32r),
                        in_=st[:],
                    )
            # K natural + bias, grouped as [head, 64 cols + ones col]
            for t_i in range(4):
                kt_glob = blk * 4 + t_i
                pn = psp.tile([128, 512], f32, tag="ps")
                for j in range(4):
                    nc.tensor.matmul(
                        pn[:], xt_sb[j][:, t_i * 128:(t_i + 1) * 128], wkt_sb[j][:],
                        start=(j == 0), stop=(j == 3), skip_group_check=True,
                    )
                ka = kap.tile([128, 4 * 65], f32r, tag="ka", name=f"kaA{kt_glob}")
                ka_sb[kt_glob] = ka
                nc.vector.tensor_add(
                    g3(ka, 65)[:, :, 0:64],
                    g3(pn[:, 0:256], 64),
                    g3(bkb_sb[:, 0:256], 64),
                )
                nc.vector.tensor_copy(
                    g3(ka, 65)[:, :, 64:65], g3(ones_r[:, 0:4], 1)
                )
                stb = stp.tile([128, 4 * 65], f32r, tag="stb")
                nc.vector.tensor_add(
                    g3(stb, 65)[:, :, 0:64],
                    g3(pn[:, 256:512], 64),
                    g3(bkb_sb[:, 256:512], 64),
                )
                nc.vector.tensor_copy(
                    g3(stb, 65)[:, :, 64:65], g3(ones_r[:, 4:8], 1)
                )
                nc.sync.dma_start(
                    out=kasp_e[kt_glob, :, 4 * 65:8 * 65].bitcast(f32r), in_=stb[:]
                )
        # QT (pre-scaled): lhsT = WQT_scaled[:, d-slice], rhs = xq block
        for qb in range(2):
            xq_sb = []
            for j in range(4):
                t = xtp.tile([128, 512], f32r, tag="xt")
                nc.sync.dma_start(
                    out=t[:],
                    in_=xq_e[j * 128:(j + 1) * 128,
                             qb * 512:(qb + 1) * 512].bitcast(f32r),
                )
                xq_sb.append(t)
            for m in range(4):
                pq = psp.tile([128, 512], f32, tag="ps")
                for j in range(4):
                    nc.tensor.matmul(
                        pq[:], wqt_sb[j][:, m * 128:(m + 1) * 128], xq_sb[j][:],
                        start=(j == 0), stop=(j == 3), skip_group_check=True,
                    )
                if m < 2:
                    nc.vector.tensor_scalar_add(
                        qt_sb[m][:, qb * 512:(qb + 1) * 512], pq[:], bqs_sb[m][:]
                    )
                else:
                    st = stp.tile([128, 512], f32r, tag="st")
                    nc.vector.tensor_scalar_add(st[:], pq[:], bqs_sb[m][:])
                    nc.sync.dma_start(
                        out=qtsp_e[m - 2, :, qb * 512:(qb + 1) * 512].bitcast(f32r),
                        in_=st[:],
                    )

        # ---- attention, two head-groups of 4 ----
        for g in range(2):
            if g == 1:
                for m in (2, 3):
                    kt_sb[m] = ktp.tile([128, S], f32r, tag="kt", name=f"ktB{m}")
                    nc.sync.dma_start(out=kt_sb[m][:],
                                      in_=ktsp_e[m - 2].bitcast(f32r))
                    qt_sb[m] = qtp.tile([128, QL], f32r, tag="qt", name=f"qtB{m}")
                    nc.sync.dma_start(out=qt_sb[m][:],
                                      in_=qtsp_e[m - 2].bitcast(f32r))
                for kt_i in range(32):
                    ka_sb[kt_i] = kap.tile([128, 4 * 65], f32r, tag="ka", name=f"kaB{kt_i}")
                    nc.sync.dma_start(
                        out=ka_sb[kt_i][:],
                        in_=kasp_e[kt_i, :, 4 * 65:8 * 65].bitcast(f32r),
                    )
            for hp in range(2):
                h0 = 4 * g + 2 * hp
                mt = h0 // 2
                ktt, qtt = kt_sb[mt], qt_sb[mt]
                # -- natural orientation: probs for the scores output --
                for qi in range(8):
                    for hh in range(2):
                        h = h0 + hh
                        po = hh * 64
                        pr = prp.tile([128, S], f32, tag="probs")
                        ac = accp.tile([128, 4], f32, tag="acc")
                        for kb in range(4):
                            pp = psp.tile([128, 1024], f32, tag="ps")
                            for hf in range(2):
                                nc.tensor.matmul(
                                    pp[:, hf * 512:(hf + 1) * 512],
                                    qtt[po:po + 64, qi * 128:(qi + 1) * 128],
                                    ktt[po:po + 64,
                                        (2 * kb + hf) * 512:(2 * kb + hf + 1) * 512],
                                    start=True, stop=True, skip_group_check=True,
                                )
                            if has_mask:
                                nc.vector.tensor_add(
                                    pp[:], pp[:],
                                    maskb_sb[:, kb * 1024:(kb + 1) * 1024],
                                )
                            nc.scalar.activation(
                                out=pr[:, kb * 1024:(kb + 1) * 1024], in_=pp[:],
                                func=AF.Exp, accum_out=ac[:, kb:kb + 1],
                            )
                        dn = accp.tile([128, 1], f32, tag="dn")
                        nc.vector.reduce_sum(dn[:], ac[:], axis=AX.X)
                        nc.vector.reciprocal(dn[:], dn[:])
                        nc.vector.tensor_scalar_mul(pr[:], pr[:], dn[:])
                        nc.sync.dma_start(
                            out=scores_e[h, qi * 128:(qi + 1) * 128, :], in_=pr[:]
                        )
                # -- transposed orientation + AV --
                for hh in range(2):
                    h = h0 + hh
                    po = hh * 64
                    hg = 2 * hp + hh  # head index within the group (0..3)
                    avq = psp.tile([65, 1024], f32, tag="ps")
                    for kt_i in range(32):
                        pt = psp.tile([128, 1024], f32, tag="ps")
                        for hf in range(2):
                            nc.tensor.matmul(
                                pt[:, hf * 512:(hf + 1) * 512],
                                ktt[po:po + 64, kt_i * 128:(kt_i + 1) * 128],
                                qtt[po:po + 64, hf * 512:(hf + 1) * 512],
                                start=True, stop=True, skip_group_check=True,
                            )
                        et = etp.tile([128, 1024], f32r, tag="et")
                        if has_mask:
                            nc.scalar.activation(out=et[:], in_=pt[:], func=AF.Exp,
                                                 bias=maskt_sb[:, kt_i:kt_i + 1])
                        else:
                            nc.scalar.activation(out=et[:], in_=pt[:], func=AF.Exp)
                        for hf in range(2):
                            nc.tensor.matmul(
                                avq[:, hf * 512:(hf + 1) * 512],
                                ka_sb[kt_i][:, hg * 65:(hg + 1) * 65],
                                et[:, hf * 512:(hf + 1) * 512],
                                start=(kt_i == 0), stop=(kt_i == 31),
                                skip_group_check=True,
                            )
                    nc.vector.tensor_copy(avt_sb[mt][po:po + 64, :], avq[0:64, :])
                    dnr = dnp.tile([65, 1024], f32, tag="dnr")
                    nc.vector.tensor_copy(dnr[64:65, :], avq[64:65, :])
                    nc.vector.reciprocal(dnr[64:65, :], dnr[64:65, :])
                    nc.sync.dma_start(out=dsp_e[h:h + 1, :], in_=dnr[64:65, :])
                # normalize this pair's avT tile by broadcast 1/denom
                bc = bcp.tile([128, QL], f32r, tag="bc")
                for hh in range(2):
                    nc.gpsimd.dma_start(
                        out=bc[hh * 64:(hh + 1) * 64, :],
                        in_=_bcast_rows(dsp_e[h0 + hh, :], 64).bitcast(f32r),
                    )
                nc.vector.tensor_mul(avt_sb[mt][:], avt_sb[mt][:], bc[:])

        # ---- output projection: outT = WOT.T @ avT + bO ----
        for m in range(4):
            for qb in range(2):
                pop = psp.tile([128, 512], f32, tag="ps")
                for j in range(4):
                    nc.tensor.matmul(
                        pop[:], wot_sb[j][:, m * 128:(m + 1) * 128],
                        avt_sb[j][:, qb * 512:(qb + 1) * 512],
                        start=(j == 0), stop=(j == 3), skip_group_check=True,
                    )
                ot = otp.tile([128, 512], f32, tag="ot")
                nc.vector.tensor_scalar_add(ot[:], pop[:], bo_sb[m][:])
                nc.sync.dma_start(
                    out=outt_e[m * 128:(m + 1) * 128, qb * 512:(qb + 1) * 512],
                    in_=ot[:],
                )

    _legalize_single_wait(nc)
    return nc


_KERNEL_CACHE = {}


def _get_kernel(has_mask: bool):
    if has_mask not in _KERNEL_CACHE:
        _KERNEL_CACHE[has_mask] = build_kernel(has_mask)
    return _KERNEL_CACHE[has_mask]


def kernel(x, attention_mask, WQ, bQ, WK, bK, WO, bO, _profile=False):
    global LAST_EXEC_NS
    from concourse.bass_utils import run_bass_kernel_spmd

    x = np.asarray(x, dtype=np.float32)
    mask = np.asarray(attention_mask, dtype=np.float32)[:, 0, 0, :]  # [B, S]
    WQ, bQ = np.asarray(WQ, np.float32), np.asarray(bQ, np.float32)
    WK, bK = np.asarray(WK, np.float32), np.asarray(bK, np.float32)
    WO, bO = np.asarray(WO, np.float32), np.asarray(bO, np.float32)

    has_mask = bool(np.any(mask))
    nc = _get_kernel(has_mask)

    wqt = np.ascontiguousarray(WQ.T) * np.float32(SCALE)
    wkt = np.ascontiguousarray(WK.T)
    wot = np.ascontiguousarray(WO.T)
    bqs = (bQ * np.float32(SCALE)).reshape(H, 1)
    bk2 = bK.reshape(H, 1)
    bo2 = bO.reshape(H, 1)
    bkb = np.ascontiguousarray(np.broadcast_to(bK, (128, H)))

    xT = [np.ascontiguousarray(x[b].T) for b in range(B)]

    in_maps = []
    for c in range(NCORES):
        b, r = c // 4, (c % 4) * QL
        m = {
            "xt": xT[b],
            "xq": np.ascontiguousarray(xT[b][:, r:r + QL]),
            "wqt": wqt, "wkt": wkt, "wot": wot,
            "bqs": bqs, "bk": bk2, "bo": bo2, "bkb": bkb,
        }
        if has_mask:
            m["maskt"] = np.ascontiguousarray(mask[b].reshape(S, 1))
            m["maskb"] = np.ascontiguousarray(np.broadcast_to(mask[b], (128, S)))
        in_maps.append(m)

    res = run_bass_kernel_spmd(nc, in_maps, list(range(NCORES)), trace=_profile)
    LAST_EXEC_NS = res.exec_time_ns

    scores = np.empty((B, NH, S, S), dtype=np.float32)
    out = np.empty((B, S, H), dtype=np.float32)
    for c in range(NCORES):
        b, r = c // 4, (c % 4) * QL
        rc = res.results[c]
        scores[b, :, r:r + QL, :] = rc["scores"]
        out[b, r:r + QL, :] = rc["outt"].T
        res.results[c] = None
    return (out, scores)


# revision 5
# speedup vs baseline: 1.0432x; 1.0432x over previous
"""MultiHeadAttention Trainium2 Bass kernel (8-core SPMD).

Reference computes (out, scores) where
  q = x @ WQ.T + bQ ; k = x @ WK.T + bK ; v = k   (source quirk: V == K)
  scores = softmax(mask + q k^T / sqrt(dh))       # [B, nh, S, S]  ~1 GiB f32
  out = (scores @ v) proj WO + bO                 # [B, S, H]

Sharding: batch x query-block. Core c handles batch b=c//4 and query rows
r=(c%4)*1024 .. +1024, all 8 heads. No cross-core communication.

Per-core device pipeline (all matmuls float32r, full PE rate at N=512):
  xT (hidden-major x) -> KT [d, keys] and K-aug [keys, d-grouped+ones-col]
  and QT [d, q] (pre-scaled by 1/sqrt(dh)).
  Natural side:    scores[q,k] -> exp (fused accum denominator) -> normalize
                   -> DMA out (the 1 GiB scores output, q-major).
  Transposed side: scores^T[k,q] -> exp -> AV matmul with K-aug as the
                   stationary operand; the appended ones-column yields the
                   transposed-side softmax denominators in the same psum.
  avT normalized via a DRAM-bounced partition-broadcast of 1/denom, then
  out^T = WOT.T @ avT + bO, DMAed back; host transposes.
"""

import sys

sys.path.insert(0, "/opt/trn_rl_repo")

from contextlib import ExitStack

import numpy as np

import concourse.bass as bass
import concourse.mybir as mybir
import concourse.tile as tile

B, S, H, NH, DH = 2, 4096, 512, 8, 64
NCORES = 8
QL = S // 4  # local query rows per core (1024)
SCALE = 1.0 / np.sqrt(DH)

f32 = mybir.dt.float32
f32r = mybir.dt.float32r
bf16 = mybir.dt.bfloat16
AF = mybir.ActivationFunctionType
AX = mybir.AxisListType

LAST_EXEC_NS = None


# --- walrus in this toolchain encodes at most ONE sem-wait per instruction;
# move extra waits onto single-wait NoOps inserted before, same engine. ---
def _legalize_single_wait(nc):
    ctr = 0
    for f in nc.m.functions:
        for bb in f.blocks:
            out = []
            changed = False
            for inst in bb.instructions:
                si = inst.sync_info
                waits = list(si.on_wait) if (si and si.on_wait) else []
                ups = list(si.on_update) if (si and si.on_update) else []
                if len(ups) > 1:
                    raise RuntimeError(f"{inst.name}: {len(ups)} sem updates")
                if len(waits) > 1:
                    changed = True
                    for w in waits[:-1]:
                        ctr += 1
                        out.append(
                            mybir.InstNoOp(
                                name=f"__legalize_wait_nop_{ctr}",
                                engine=inst.engine,
                                sync_info=mybir.SyncInfo(on_wait=[w], on_update=[]),
                            )
                        )
                    inst.sync_info = mybir.SyncInfo(on_wait=waits[-1:], on_update=ups)
                out.append(inst)
            if changed:
                try:
                    bb.instructions = out
                except Exception:
                    bb.instructions.clear()
                    for i in out:
                        bb.instructions.append(i)


def _bcast_rows(dram_ap, nrows):
    """Partition-broadcast AP: replicate a 1D DRAM row across nrows partitions."""
    return bass.AP(
        tensor=dram_ap.tensor, offset=dram_ap.offset, ap=[[0, nrows]] + dram_ap.ap
    )


def build_kernel(has_mask: bool):
    nc = bass.Bass()

    xt_e = nc.declare_dram_parameter("xt", [H, S], f32, isOutput=False)
    xq_e = nc.declare_dram_parameter("xq", [H, QL], f32, isOutput=False)
    wqt_e = nc.declare_dram_parameter("wqt", [H, H], f32, isOutput=False)
    wkt_e = nc.declare_dram_parameter("wkt", [H, H], f32, isOutput=False)
    wot_e = nc.declare_dram_parameter("wot", [H, H], f32, isOutput=False)
    bqs_e = nc.declare_dram_parameter("bqs", [H, 1], f32, isOutput=False)
    bk_e = nc.declare_dram_parameter("bk", [H, 1], f32, isOutput=False)
    bo_e = nc.declare_dram_parameter("bo", [H, 1], f32, isOutput=False)
    bkb_e = nc.declare_dram_parameter("bkb", [128, H], f32, isOutput=False)
    if has_mask:
        maskt_e = nc.declare_dram_parameter("maskt", [S, 1], f32, isOutput=False)
        maskb_e = nc.declare_dram_parameter("maskb", [128, S], f32, isOutput=False)

    scores_e = nc.declare_dram_parameter("scores", [NH, QL, S], f32, isOutput=True)
    outt_e = nc.declare_dram_parameter("outt", [H, QL], f32, isOutput=True)
    # spill space (device-internal scratch; declared as outputs because the
    # PJRT path only materializes IO tensors)
    ktsp_e = nc.declare_dram_parameter("ktsp", [2, 128, S], f32, isOutput=True)
    kasp_e = nc.declare_dram_parameter("kasp", [32, 128, 8 * 65], f32, isOutput=True)
    qtsp_e = nc.declare_dram_parameter("qtsp", [2, 128, QL], f32, isOutput=True)
    dsp_e = nc.declare_dram_parameter("dsp", [NH, QL], f32, isOutput=True)

    with tile.TileContext(nc) as tc, ExitStack() as ctx:
        consts = ctx.enter_context(tc.tile_pool(name="consts", bufs=1))
        ktp = ctx.enter_context(tc.tile_pool(name="ktp", bufs=2))
        kap = ctx.enter_context(tc.tile_pool(name="kap", bufs=32))
        qtp = ctx.enter_context(tc.tile_pool(name="qtp", bufs=2))
        prp = ctx.enter_context(tc.tile_pool(name="prp", bufs=2))
        etp = ctx.enter_context(tc.tile_pool(name="etp", bufs=3))
        avp = ctx.enter_context(tc.tile_pool(name="avp", bufs=4))
        bcp = ctx.enter_context(tc.tile_pool(name="bcp", bufs=1))
        dnp = ctx.enter_context(tc.tile_pool(name="dnp", bufs=1))
        otp = ctx.enter_context(tc.tile_pool(name="otp", bufs=2))
        accp = ctx.enter_context(tc.tile_pool(name="accp", bufs=2))
        stp = ctx.enter_context(tc.tile_pool(name="stp", bufs=2))
        xtp = ctx.enter_context(tc.tile_pool(name="xtp", bufs=6))
        psp = ctx.enter_context(tc.tile_pool(name="psp", bufs=4, space="PSUM"))

        # ---- constants ----
        wkt_sb, wqt_sb, wot_sb = [], [], []
        for j in range(4):
            w = consts.tile([128, H], f32r, tag=f"wkt{j}")
            nc.sync.dma_start(out=w[:], in_=wkt_e[j * 128:(j + 1) * 128, :].bitcast(f32r))
            wkt_sb.append(w)
        for j in range(4):
            w = consts.tile([128, H], f32r, tag=f"wqt{j}")
            nc.sync.dma_start(out=w[:], in_=wqt_e[j * 128:(j + 1) * 128, :].bitcast(f32r))
            wqt_sb.append(w)
        for j in range(4):
            w = consts.tile([128, H], f32r, tag=f"wot{j}")
            nc.sync.dma_start(out=w[:], in_=wot_e[j * 128:(j + 1) * 128, :].bitcast(f32r))
            wot_sb.append(w)
        bqs_sb, bk_sb, bo_sb = [], [], []
        for m in range(4):
            t = consts.tile([128, 1], f32, tag=f"bqs{m}")
            nc.gpsimd.dma_start(out=t[:], in_=bqs_e[m * 128:(m + 1) * 128, :])
            bqs_sb.append(t)
            t = consts.tile([128, 1], f32, tag=f"bk{m}")
            nc.gpsimd.dma_start(out=t[:], in_=bk_e[m * 128:(m + 1) * 128, :])
            bk_sb.append(t)
            t = consts.tile([128, 1], f32, tag=f"bo{m}")
            nc.gpsimd.dma_start(out=t[:], in_=bo_e[m * 128:(m + 1) * 128, :])
            bo_sb.append(t)
        bkb_sb = consts.tile([128, H], f32, tag="bkb")
        nc.gpsimd.dma_start(out=bkb_sb[:], in_=bkb_e[:])
        ones_f = consts.tile([128, 8], f32, tag="ones_f")
        nc.vector.memset(ones_f[:], 1.0)
        ones_r = consts.tile([128, 8], f32r, tag="ones_r")
        nc.vector.tensor_copy(ones_r[:], ones_f[:])
        if has_mask:
            maskt_sb = consts.tile([128, 32], f32, tag="maskt")
            nc.gpsimd.dma_start(
                out=maskt_sb[:],
                in_=maskt_e[:, 0].rearrange("(j p) -> p j", p=128),
            )
            maskb_sb = consts.tile([128, S], f32, tag="maskb")
            nc.gpsimd.dma_start(out=maskb_sb[:], in_=maskb_e[:])

        # ---- long-lived tensors ----
        kt_sb = [None] * 4   # KT m-tiles [128 d, S keys] f32r (m>=2 spilled)
        qt_sb = [None] * 4   # QT m-tiles [128 d, QL] f32r
        ka_sb = [None] * 32  # K-aug key-tiles [128 keys, 8*65] (split by head half)
        kt_sb[0] = ktp.tile([128, S], bf16, tag="kt", name="kt0")
        kt_sb[1] = ktp.tile([128, S], bf16, tag="kt", name="kt1")
        qt_sb[0] = qtp.tile([128, QL], bf16, tag="qt", name="qt0")
        qt_sb[1] = qtp.tile([128, QL], bf16, tag="qt", name="qt1")
        avt_sb = [avp.tile([128, QL], f32r, tag="avt", name=f"avt{i}") for i in range(4)]

        def g3(ap, c):
            return ap.rearrange("p (h c) -> p h c", c=c)

        # ---- phase P: projections ----
        for blk in range(8):
            xt_sb = []
            for j in range(4):
                t = xtp.tile([128, 512], f32r, tag="xt")
                nc.sync.dma_start(
                    out=t[:],
                    in_=xt_e[j * 128:(j + 1) * 128,
                             blk * 512:(blk + 1) * 512].bitcast(f32r),
                )
                xt_sb.append(t)
            # KT[d, keys]: lhsT = WKT[:, d-slice], rhs = xT block
            for m in range(4):
                pk = psp.tile([128, 512], f32, tag="ps")
                for j in range(4):
                    nc.tensor.matmul(
                        pk[:], wkt_sb[j][:, m * 128:(m + 1) * 128], xt_sb[j][:],
                        start=(j == 0), stop=(j == 3), skip_group_check=True,
                    )
                if m < 2:
                    nc.vector.tensor_scalar_add(
                        kt_sb[m][:, blk * 512:(blk + 1) * 512], pk[:], bk_sb[m][:]
                    )
                else:
                    st = stp.tile([128, 512], bf16, tag="st")
                    nc.vector.tensor_scalar_add(st[:], pk[:], bk_sb[m][:])
                    nc.sync.dma_start(
                        out=ktsp_e[m - 2, :, blk * 256:(blk + 1) * 256].bitcast(bf16),
                        in_=st[:],
                    )
            # K natural + bias, grouped as [head, 64 cols + ones col]
            for t_i in range(4):
                kt_glob = blk * 4 + t_i
                pn = psp.tile([128, 512], f32, tag="ps")
                for j in range(4):
                    nc.tensor.matmul(
                        pn[:], xt_sb[j][:, t_i * 128:(t_i + 1) * 128], wkt_sb[j][:],
                        start=(j == 0), stop=(j == 3), skip_group_check=True,
                    )
                ka = kap.tile([128, 4 * 65], f32r, tag="ka", name=f"kaA{kt_glob}")
                ka_sb[kt_glob] = ka
                nc.vector.tensor_add(
                    g3(ka, 65)[:, :, 0:64],
                    g3(pn[:, 0:256], 64),
                    g3(bkb_sb[:, 0:256], 64),
                )
                nc.vector.tensor_copy(
                    g3(ka, 65)[:, :, 64:65], g3(ones_r[:, 0:4], 1)
                )
                stb = stp.tile([128, 4 * 65], f32r, tag="stb")
                nc.vector.tensor_add(
                    g3(stb, 65)[:, :, 0:64],
                    g3(pn[:, 256:512], 64),
                    g3(bkb_sb[:, 256:512], 64),
                )
                nc.vector.tensor_copy(
                    g3(stb, 65)[:, :, 64:65], g3(ones_r[:, 4:8], 1)
                )
                nc.sync.dma_start(
                    out=kasp_e[kt_glob, :, 4 * 65:8 * 65].bitcast(f32r), in_=stb[:]
                )
        # QT (pre-scaled): lhsT = WQT_scaled[:, d-slice], rhs = xq block
        for qb in range(2):
            xq_sb = []
            for j in range(4):
                t = xtp.tile([128, 512], f32r, tag="xt")
                nc.sync.dma_start(
                    out=t[:],
                    in_=xq_e[j * 128:(j + 1) * 128,
                             qb * 512:(qb + 1) * 512].bitcast(f32r),
                )
                xq_sb.append(t)
            for m in range(4):
                pq = psp.tile([128, 512], f32, tag="ps")
                for j in range(4):
                    nc.tensor.matmul(
                        pq[:], wqt_sb[j][:, m * 128:(m + 1) * 128], xq_sb[j][:],
                        start=(j == 0), stop=(j == 3), skip_group_check=True,
                    )
                if m < 2:
                    nc.vector.tensor_scalar_add(
                        qt_sb[m][:, qb * 512:(qb + 1) * 512], pq[:], bqs_sb[m][:]
                    )
                else:
                    st = stp.tile([128, 512], bf16, tag="st")
                    nc.vector.tensor_scalar_add(st[:], pq[:], bqs_sb[m][:])
                    nc.sync.dma_start(
                        out=qtsp_e[m - 2, :, qb * 256:(qb + 1) * 256].bitcast(bf16),
                        in_=st[:],
                    )

        # ---- attention, two head-groups of 4 ----
        for g in range(2):
            if g == 1:
                for m in (2, 3):
                    kt_sb[m] = ktp.tile([128, S], bf16, tag="kt", name=f"ktB{m}")
                    nc.sync.dma_start(out=kt_sb[m][:],
                                      in_=ktsp_e[m - 2, :, 0:S // 2].bitcast(bf16))
                    qt_sb[m] = qtp.tile([128, QL], bf16, tag="qt", name=f"qtB{m}")
                    nc.sync.dma_start(out=qt_sb[m][:],
                                      in_=qtsp_e[m - 2, :, 0:QL // 2].bitcast(bf16))
                for kt_i in range(32):
                    ka_sb[kt_i] = kap.tile([128, 4 * 65], f32r, tag="ka", name=f"kaB{kt_i}")
                    nc.sync.dma_start(
                        out=ka_sb[kt_i][:],
                        in_=kasp_e[kt_i, :, 4 * 65:8 * 65].bitcast(f32r),
                    )
            for hp in range(2):
                h0 = 4 * g + 2 * hp
                mt = h0 // 2
                ktt, qtt = kt_sb[mt], qt_sb[mt]
                # -- natural orientation: probs for the scores output --
                for qi in range(8):
                    for hh in range(2):
                        h = h0 + hh
                        po = hh * 64
                        pr = prp.tile([128, S], f32, tag="probs")
                        ac = accp.tile([128, 4], f32, tag="acc")
                        for kb in range(4):
                            pp = psp.tile([128, 1024], f32, tag="ps")
                            for hf in range(2):
                                nc.tensor.matmul(
                                    pp[:, hf * 512:(hf + 1) * 512],
                                    qtt[po:po + 64, qi * 128:(qi + 1) * 128],
                                    ktt[po:po + 64,
                                        (2 * kb + hf) * 512:(2 * kb + hf + 1) * 512],
                                    start=True, stop=True, skip_group_check=True,
                                )
                            if has_mask:
                                nc.vector.tensor_add(
                                    pp[:], pp[:],
                                    maskb_sb[:, kb * 1024:(kb + 1) * 1024],
                                )
                            nc.scalar.activation(
                                out=pr[:, kb * 1024:(kb + 1) * 1024], in_=pp[:],
                                func=AF.Exp, accum_out=ac[:, kb:kb + 1],
                            )
                        dn = accp.tile([128, 1], f32, tag="dn")
                        nc.vector.reduce_sum(dn[:], ac[:], axis=AX.X)
                        nc.vector.reciprocal(dn[:], dn[:])
                        nc.vector.tensor_scalar_mul(pr[:], pr[:], dn[:])
                        nc.sync.dma_start(
                            out=scores_e[h, qi * 128:(qi + 1) * 128, :], in_=pr[:]
                        )
                # -- transposed orientation + AV --
                for hh in range(2):
                    h = h0 + hh
                    po = hh * 64
                    hg = 2 * hp + hh  # head index within the group (0..3)
                    avq = psp.tile([65, 1024], f32, tag="ps")
                    for kt_i in range(32):
                        pt = psp.tile([128, 1024], f32, tag="ps")
                        for hf in range(2):
                            nc.tensor.matmul(
                                pt[:, hf * 512:(hf + 1) * 512],
                                ktt[po:po + 64, kt_i * 128:(kt_i + 1) * 128],
                                qtt[po:po + 64, hf * 512:(hf + 1) * 512],
                                start=True, stop=True, skip_group_check=True,
                            )
                        et = etp.tile([128, 1024], f32r, tag="et")
                        if has_mask:
                            nc.scalar.activation(out=et[:], in_=pt[:], func=AF.Exp,
                                                 bias=maskt_sb[:, kt_i:kt_i + 1])
                        else:
                            nc.scalar.activation(out=et[:], in_=pt[:], func=AF.Exp)
                        for hf in range(2):
                            nc.tensor.matmul(
                                avq[:, hf * 512:(hf + 1) * 512],
                                ka_sb[kt_i][:, hg * 65:(hg + 1) * 65],
                                et[:, hf * 512:(hf + 1) * 512],
                                start=(kt_i == 0), stop=(kt_i == 31),
                                skip_group_check=True,
                            )
                    nc.vector.tensor_copy(avt_sb[mt][po:po + 64, :], avq[0:64, :])
                    dnr = dnp.tile([65, 1024], f32, tag="dnr")
                    nc.vector.tensor_copy(dnr[64:65, :], avq[64:65, :])
                    nc.vector.reciprocal(dnr[64:65, :], dnr[64:65, :])
                    nc.sync.dma_start(out=dsp_e[h:h + 1, :], in_=dnr[64:65, :])
                # normalize this pair's avT tile by broadcast 1/denom
                bc = bcp.tile([128, QL], f32r, tag="bc")
                for hh in range(2):
                    nc.gpsimd.dma_start(
                        out=bc[hh * 64:(hh + 1) * 64, :],
                        in_=_bcast_rows(dsp_e[h0 + hh, :], 64).bitcast(f32r),
                    )
                nc.vector.tensor_mul(avt_sb[mt][:], avt_sb[mt][:], bc[:])

        # ---- output projection: outT = WOT.T @ avT + bO ----
        for m in range(4):
            for qb in range(2):
                pop = psp.tile([128, 512], f32, tag="ps")
                for j in range(4):
                    nc.tensor.matmul(
                        pop[:], wot_sb[j][:, m * 128:(m + 1) * 128],
                        avt_sb[j][:, qb * 512:(qb + 1) * 512],
                        start=(j == 0), stop=(j == 3), skip_group_check=True,
                    )
                ot = otp.tile([128, 512], f32, tag="ot")
                nc.vector.tensor_scalar_add(ot[:], pop[:], bo_sb[m][:])
                nc.sync.dma_start(
                    out=outt_e[m * 128:(m + 1) * 128, qb * 512:(qb + 1) * 512],
                    in_=ot[:],
                )

    _legalize_single_wait(nc)
    return nc


_KERNEL_CACHE = {}


def _get_kernel(has_mask: bool):
    if has_mask not in _KERNEL_CACHE:
        _KERNEL_CACHE[has_mask] = build_kernel(has_mask)
    return _KERNEL_CACHE[has_mask]


def kernel(x, attention_mask, WQ, bQ, WK, bK, WO, bO, _profile=False):
    global LAST_EXEC_NS
    from concourse.bass_utils import run_bass_kernel_spmd

    x = np.asarray(x, dtype=np.float32)
    mask = np.asarray(attention_mask, dtype=np.float32)[:, 0, 0, :]  # [B, S]
    WQ, bQ = np.asarray(WQ, np.float32), np.asarray(bQ, np.float32)
    WK, bK = np.asarray(WK, np.float32), np.asarray(bK, np.float32)
    WO, bO = np.asarray(WO, np.float32), np.asarray(bO, np.float32)

    has_mask = bool(np.any(mask))
    nc = _get_kernel(has_mask)

    wqt = np.ascontiguousarray(WQ.T) * np.float32(SCALE)
    wkt = np.ascontiguousarray(WK.T)
    wot = np.ascontiguousarray(WO.T)
    bqs = (bQ * np.float32(SCALE)).reshape(H, 1)
    bk2 = bK.reshape(H, 1)
    bo2 = bO.reshape(H, 1)
    bkb = np.ascontiguousarray(np.broadcast_to(bK, (128, H)))

    xT = [np.ascontiguousarray(x[b].T) for b in range(B)]

    in_maps = []
    for c in range(NCORES):
        b, r = c // 4, (c % 4) * QL
        m = {
            "xt": xT[b],
            "xq": np.ascontiguousarray(xT[b][:, r:r + QL]),
            "wqt": wqt, "wkt": wkt, "wot": wot,
            "bqs": bqs, "bk": bk2, "bo": bo2, "bkb": bkb,
        }
        if has_mask:
            m["maskt"] = np.ascontiguousarray(mask[b].reshape(S, 1))
            m["maskb"] = np.ascontiguousarray(np.broadcast_to(mask[b], (128, S)))
        in_maps.append(m)

    res = run_bass_kernel_spmd(nc, in_maps, list(range(NCORES)), trace=_profile)
    LAST_EXEC_NS = res.exec_time_ns

    scores = np.empty((B, NH, S, S), dtype=np.float32)
    out = np.empty((B, S, H), dtype=np.float32)
    for c in range(NCORES):
        b, r = c // 4, (c % 4) * QL
        rc = res.results[c]
        scores[b, :, r:r + QL, :] = rc["scores"]
        out[b, r:r + QL, :] = rc["outt"].T
        res.results[c] = None
    return (out, scores)
